# revision 16
# baseline (speedup 1.0000x reference)
"""Trainium2 Bass kernel for nn_CausalSelfAttention_22127671509246.

Full (unsharded) inputs in, full output out. Internally shards across 8
NeuronCores: core c handles batch b = c // 4 and head group g = c % 4
(heads 4g..4g+3, i.e. a 256-wide slice of the QKV output channels).

Per-core compute (all matmuls bf16, f32 PSUM accumulation):
  - Q^T, K^T projections in channel-major layout [256, 2048]
  - V projection in row-major layout with a ones column appended per head
    (so the PV matmul also produces the softmax denominator)
  - attention processed in head PAIRS (partition bases 0 and 64) so the
    K=64 QK matmuls overlap in distinct PE row groups
  - attT[k, q] = K^T_h.T @ Q^T_h -> exp(attT / 8) -> SBUF-resident bf16 ax
    buffer -> PV accumulated per 128-row q tile over all k tiles ->
    normalize by the ones-column denominator.

Schedule (v2): the kernel is paced by the PE; ScalarE alone cannot keep up
with the 16.8M-element exp stream (would be ~130us), so exp tiles are split
between ScalarE (activation Exp) and the DVE (Schraudolph bit-trick:
int16 = logit * 16/ln2 + (127*128 - C), whose bit pattern IS bf16 exp;
~1.8% rms on a ~37% slice of tiles).  Projections are not a separate
phase: K/Q ct0 are pipelined against the DMA at the start so attention
begins at ~5us, and all remaining projection matmuls are stuffed into the
attention blocks' PE stream.  This keeps the PE dense end-to-end so the
HAM clock gate stays at 8/8 (2.4 GHz) instead of oscillating.
Softmax max-subtraction is skipped: logits are ~N(0,1) (max |logit| ~ 7),
so exp never overflows in f32 and softmax is shift-invariant.
"""

import os
import sys
import types

sys.path.insert(0, "/opt/trn_rl_repo")

import numpy as np
import ml_dtypes

import concourse.bass as bass
import concourse.bacc as bacc
import concourse.mybir as mybir
import concourse.tile as tile
from concourse.bass import ts

B, S, D = 2, 2048, 1024
H, HD = 16, 64
N_CORES = 8
C = 256           # output channels per core (4 heads)
CT = C // 128     # channel tiles per core
KD = D // 128     # contraction chunks for the projections
SC = S // 512     # 512-wide column chunks of S
STL = S // 128    # 128-row tiles of S
HPC = 4           # heads per core
SCALE = 1.0 / np.sqrt(HD)

# Schraudolph exp on DVE: i16 = raw_logit * (SCALE*128/ln2) + (127*128 - C0)
SCH_C0 = 7.5
SCH_A = float(SCALE * 128.0 / np.log(2.0))
SCH_B = float(127.0 * 128.0 - SCH_C0)
# kt tiles handled by the DVE (rest on ScalarE); spread so each kp step
# keeps both engines fed.
DVE_KT = frozenset((1, 4, 6, 9, 11, 14))
DVE_KT_LATE = frozenset((1, 3, 5, 7, 9, 11, 13, 15))

F32 = mybir.dt.float32
BF16 = mybir.dt.bfloat16
I16 = mybir.dt.int16

_compiled = {}


def _install_ntff_hook():
    """Optional: register the axon NTFF profiling hook if the image lacks it."""
    if "antenv.axon_hooks" in sys.modules:
        return
    try:
        import trn_agent_boot.trn_boot as tb

        mod = types.ModuleType("antenv.axon_hooks")
        hook = tb._ntff_profile_via_ctypes("/opt/axon/libaxon_pjrt.so")
        mod.get_axon_ntff_profile_hook = lambda: hook
        mod.set_axon_ntff_profile_hook = lambda h: None
        sys.modules["antenv.axon_hooks"] = mod
    except Exception:
        pass


def _emit(tc, ctx):
    nc = tc.nc
    xT = nc.dram_tensor("xT", [D, S], BF16, kind="ExternalInput").ap()
    wq = nc.dram_tensor("wq", [D, C], BF16, kind="ExternalInput").ap()
    wk = nc.dram_tensor("wk", [D, C], BF16, kind="ExternalInput").ap()
    wv = nc.dram_tensor("wv", [D, C], BF16, kind="ExternalInput").ap()
    bq = nc.dram_tensor("bq", [C], F32, kind="ExternalInput").ap()
    bk = nc.dram_tensor("bk", [C], F32, kind="ExternalInput").ap()
    bv = nc.dram_tensor("bv", [C], F32, kind="ExternalInput").ap()
    y = nc.dram_tensor("y", [S, C], F32, kind="ExternalOutput").ap()

    singles = ctx.enter_context(tc.tile_pool(name="singles", bufs=1))
    ax_pool = ctx.enter_context(tc.tile_pool(name="ax", bufs=3))
    yout_pool = ctx.enter_context(tc.tile_pool(name="yout", bufs=3))
    recip_pool = ctx.enter_context(tc.tile_pool(name="recip", bufs=4))
    ps_pool = ctx.enter_context(tc.tile_pool(name="ps", bufs=3, space="PSUM"))
    psy_pool = ctx.enter_context(tc.tile_pool(name="psy", bufs=1, space="PSUM"))

    # ---- SBUF tiles ----
    xT_r = xT.rearrange("(o p) s -> p o s", p=128)
    xT_sb = singles.tile([128, KD, S], BF16)
    w_sbs = {}
    w_sbs["k"] = singles.tile([128, KD, C], BF16, tag="wk", name="wk_sb")
    w_sbs["q"] = singles.tile([128, KD, C], BF16, tag="wq", name="wq_sb")
    w_sbs["v"] = singles.tile([128, KD, C], BF16, tag="wv", name="wv_sb")
    bq_sb = singles.tile([128, CT], F32, tag="bq")
    bk_sb = singles.tile([128, CT], F32, tag="bk")
    wq_r = wq.rearrange("(o p) c -> p o c", p=128)
    wk_r = wk.rearrange("(o p) c -> p o c", p=128)
    wv_r = wv.rearrange("(o p) c -> p o c", p=128)

    # DMA order = arrival order on the sync queue.  Weights go as single
    # full-tensor transfers (4KB per partition line = full DMA bandwidth);
    # xT sc0 as small per-kd chunks (latency: the prologue projections
    # consume them one by one), the rest of xT as per-kd 1536-wide chunks
    # (3KB lines).
    nc.sync.dma_start(w_sbs["k"][:], wk_r)
    nc.sync.dma_start(bk_sb[:], bk.rearrange("(o p) -> p o", p=128))
    for kd in range(KD):
        nc.sync.dma_start(xT_sb[:, kd, 0:512], xT_r[:, kd, 0:512])
        if kd == 1:
            nc.sync.dma_start(w_sbs["q"][:], wq_r)
            nc.sync.dma_start(bq_sb[:], bq.rearrange("(o p) -> p o", p=128))
    nc.sync.dma_start(w_sbs["v"][:], wv_r)
    for kd in range(KD):
        nc.sync.dma_start(xT_sb[:, kd, 512:2048], xT_r[:, kd, 512:2048])
    # bv broadcast across partitions (DMA with partition step 0)
    bv_bc = singles.tile([128, C], F32, tag="bvbc")
    bv_bcast_ap = bass.AP(tensor=bv.tensor, offset=bv.offset,
                          ap=[[0, 128]] + list(bv.ap))
    nc.gpsimd.dma_start(out=bv_bc[:], in_=bv_bcast_ap)

    # V with a ones column appended per head: [128, s_tile, head, 65]
    v_sb = singles.tile([128, STL, HPC, HD + 1], BF16, tag="vones")
    nc.vector.memset(v_sb[:, :, :, HD], 1.0)

    qt_sb = singles.tile([128, CT, S], BF16, tag="qt")
    kt_sb = singles.tile([128, CT, S], BF16, tag="kt")

    # HAM warmup: junk matmuls on a zeroed scratch tile into the (not yet
    # used) y0 PSUM bank.  They have no DMA deps, so they keep the PE busy
    # while the prologue projections wait on input DMA — otherwise the PE
    # idles in ~3us chunks and the clock gate holds it at 1.2 GHz for the
    # first ~50us.
    warm_sb = singles.tile([128, 512], BF16, tag="warm")
    nc.vector.memset(warm_sb[:], 0.0)
    warm_ps = psy_pool.tile([128, 4, HD + 1], F32, tag="y0", name="warm_ps")

    def dummies(n):
        for _ in range(n):
            nc.tensor.matmul(
                warm_ps[:], lhsT=warm_sb[:, 0:128], rhs=warm_sb[:, 0:260],
                start=True, stop=True,
            )

    # ---- projection groups (8 matmuls + 1 bias op each) ----
    def proj_qk(which, ct, sc):
        w_sb = w_sbs[which]
        dst = qt_sb if which == "q" else kt_sb
        bias = bq_sb if which == "q" else bk_sb
        ps = ps_pool.tile([128, 1024], F32, tag="qk", name="ps_proj")
        for kd in range(KD):
            nc.tensor.matmul(
                ps[:, 0:512],
                lhsT=w_sb[:, kd, ts(ct, 128)],
                rhs=xT_sb[:, kd, ts(sc, 512)],
                start=(kd == 0),
                stop=(kd == KD - 1),
            )
        nc.vector.tensor_scalar_add(
            dst[:, ct, ts(sc, 512)], ps[:, 0:512], bias[:, ct : ct + 1]
        )

    def proj_v(st):
        # V projections run only in block 0, when the y PSUM banks are
        # still idle — use them instead of stealing a QK-ring slot (the
        # ring slot would be held until the bias-add clears the DVE FIFO,
        # stalling QK allocation and starving the exp engines).
        ps = psy_pool.tile([128, 4, HD + 1], F32, tag=f"y{st % 2}",
                           name="vps")
        flat = ps[:].rearrange("p a b -> p (a b)")
        for kd in range(KD):
            nc.tensor.matmul(
                flat[:, 0:C],
                lhsT=xT_sb[:, kd, ts(st, 128)],
                rhs=w_sbs["v"][:, kd, :],
                start=(kd == 0),
                stop=(kd == KD - 1),
            )
        nc.vector.tensor_tensor(
            v_sb[:, st, :, 0:HD],
            flat[:, 0:C].rearrange("p (h d) -> p h d", h=HPC),
            bv_bc.rearrange("p (h d) -> p h d", h=HPC),
            mybir.AluOpType.add,
        )

    # ---- attention ----
    blocks = [(pair, qc) for pair in range(HPC // 2) for qc in range(SC)]

    def qk_exp_block(pair, qc, ax_tile, pre_steps=(), late=False):
        """Per kp step: 4 QK matmuls (head pair in distinct PE row groups),
        then 2 exps routed to ScalarE or DVE.  pre_steps[s] = closures to
        emit before step s (late projection groups for block 0)."""
        ct = pair
        for kp in range(STL // 2):
            if kp < len(pre_steps):
                for fn in pre_steps[kp]:
                    fn()
            tiles = []
            for u in range(2):
                ps = ps_pool.tile([128, 1024], F32, tag="qk", name="ps_att")
                tiles.append(ps)
            for u in range(2):
                kt = 2 * kp + u
                for hh in range(2):
                    p0 = hh * 64
                    nc.tensor.matmul(
                        tiles[u][:, ts(hh, 512)],
                        lhsT=kt_sb[p0 : p0 + 64, ct, ts(kt, 128)],
                        rhs=qt_sb[p0 : p0 + 64, ct, ts(qc, 512)],
                        start=True,
                        stop=True,
                    )
            for u in range(2):
                kt = 2 * kp + u
                if kt in (DVE_KT_LATE if late else DVE_KT):
                    nc.vector.tensor_scalar(
                        ax_tile[:, kt, :].bitcast(I16), tiles[u][:],
                        SCH_A, SCH_B,
                        mybir.AluOpType.mult, mybir.AluOpType.add,
                    )
                else:
                    nc.scalar.activation(
                        ax_tile[:, kt, :], tiles[u][:],
                        mybir.ActivationFunctionType.Exp, scale=SCALE,
                    )
            yield

    def pv_mms(pair, qc, ax_tile, y_ps):
        """PV matmul closures (hh/j-major, kt innermost: PSUM accumulation
        groups must stay contiguous — interleaving them corrupts)."""
        mms = []
        for hh in range(2):
            h = 2 * pair + hh
            for j in range(4):
                for kt in range(STL):
                    def mm(hh=hh, h=h, j=j, kt=kt):
                        nc.tensor.matmul(
                            y_ps[hh][:, j, :],
                            lhsT=ax_tile[:, kt,
                                         hh * 512 + j * 128
                                         : hh * 512 + (j + 1) * 128],
                            rhs=v_sb[:, kt, h, :],
                            start=(kt == 0),
                            stop=(kt == STL - 1),
                        )
                    mms.append(mm)
        return mms

    def epilogue(pair, qc, y_ps):
        yo = yout_pool.tile([128, 4, 2, HD], F32, tag="yo", name="yo")
        for hh in range(2):
            rc = recip_pool.tile([128, 4], F32, tag="rc", name="rc")
            nc.vector.reciprocal(rc[:], y_ps[hh][:, :, HD])
            nc.vector.tensor_tensor(
                yo[:, :, hh, :],
                y_ps[hh][:, :, 0:HD],
                rc[:, :, None].to_broadcast((128, 4, HD)),
                mybir.AluOpType.mult,
            )
        nc.sync.dma_start(
            y[ts(qc, 512), ts(pair, 2 * HD)].rearrange(
                "(j p) c -> p j c", p=128),
            yo[:].rearrange("p j h d -> p j (h d)"),
        )

    # ---- schedule ----
    # Prologue: K/Q ct0 sc0 so block (0,0) can start immediately; dummy
    # matmuls ahead of each DMA-gated projection matmul keep the PE warm.
    dummies(8)

    def proj_qk_warm(which, ct, sc, per_kd):
        w_sb = w_sbs[which]
        dst = qt_sb if which == "q" else kt_sb
        bias = bq_sb if which == "q" else bk_sb
        ps = ps_pool.tile([128, 1024], F32, tag="qk", name="ps_proj")
        for kd in range(KD):
            dummies(per_kd)
            nc.tensor.matmul(
                ps[:, 0:512],
                lhsT=w_sb[:, kd, ts(ct, 128)],
                rhs=xT_sb[:, kd, ts(sc, 512)],
                start=(kd == 0),
                stop=(kd == KD - 1),
            )
        nc.vector.tensor_scalar_add(
            dst[:, ct, ts(sc, 512)], ps[:, 0:512], bias[:, ct : ct + 1]
        )

    proj_qk_warm("k", 0, 0, 2)
    proj_qk_warm("q", 0, 0, 1)

    # Late projection groups stuffed into the attention blocks' PE stream.
    # Need-by: qt[ct0,sc_j] before block j; V s-tiles 0..3 before block 1
    # step 0, 4..7 before step 2, ... (pv is kt-major, clumps at steps
    # 0/2/4/6); kt[ct1]+qt[ct1,sc0] before block 4; qt[ct1,sc_j] before
    # block 4+j.
    stuff = {}
    # block 0: remaining K ct0 groups ride ahead of their QK consumers
    # (kp step s consumes kt tiles 2s,2s+1 -> sc group (2s+1)//4).
    # V groups use the psy banks, so only proj_qk calls steal a QK-ring
    # slot; keep those <=2 per block and >=4 steps apart.
    stuff[(0, 0)] = [lambda: proj_qk("k", 0, 1)]
    stuff[(0, 2)] = [lambda: proj_qk("k", 0, 2)]
    stuff[(0, 3)] = [lambda: proj_v(0), lambda: proj_v(1),
                     lambda: proj_v(2)]
    stuff[(0, 4)] = [lambda: proj_qk("k", 0, 3),
                     lambda: proj_v(3), lambda: proj_v(4),
                     lambda: proj_v(5)]
    stuff[(0, 5)] = [lambda: proj_v(6), lambda: proj_v(7),
                     lambda: proj_v(8)]
    stuff[(0, 6)] = [lambda: proj_v(9), lambda: proj_v(10),
                     lambda: proj_v(11), lambda: proj_v(12)]
    stuff[(0, 7)] = [lambda: proj_qk("q", 0, 1),
                     lambda: proj_v(13), lambda: proj_v(14),
                     lambda: proj_v(15)]
    stuff[(1, 2)] = [lambda: proj_qk("q", 0, 2)]
    stuff[(1, 6)] = [lambda: proj_qk("k", 1, 0)]
    stuff[(2, 1)] = [lambda: proj_qk("q", 0, 3)]
    stuff[(2, 5)] = [lambda: proj_qk("k", 1, 1)]
    stuff[(3, 1)] = [lambda: proj_qk("k", 1, 2)]
    stuff[(3, 5)] = [lambda: proj_qk("q", 1, 0)]
    stuff[(4, 1)] = [lambda: proj_qk("k", 1, 3)]
    stuff[(4, 5)] = [lambda: proj_qk("q", 1, 1)]
    stuff[(5, 1)] = [lambda: proj_qk("q", 1, 2)]
    stuff[(6, 1)] = [lambda: proj_qk("q", 1, 3)]

    def alloc_y():
        return [psy_pool.tile([128, 4, HD + 1], F32, tag=f"y{hh}",
                              name=f"y{hh}") for hh in range(2)]

    prev = None  # (pair, qc, ax_tile)
    prev_y = None
    for i, (pair, qc) in enumerate(blocks):
        ax_tile = ax_pool.tile([128, STL, 1024], BF16, tag="ax", name="ax")
        if prev:
            prev_y = alloc_y()
            pv_prev = pv_mms(prev[0], prev[1], prev[2], prev_y)
        else:
            pv_prev = []
        assert len(pv_prev) in (0, 128)
        step = 0
        gen = qk_exp_block(pair, qc, ax_tile, late=(i == len(blocks) - 1))
        while True:
            for fn in stuff.get((i, step), ()):
                fn()
            if i == 0 and step < 8:
                dummies(1)  # block 0 is DMA-paced; keep the PE warm
            # PV matmuls of the previous block, 32 per two kp steps,
            # starting at step 2 so the previous block's trailing exps
            # have slack (the first PV group reads every kt tile).
            if step in (2, 4, 6):
                for mm in pv_prev[16 * (step - 2) : 16 * step]:
                    mm()
            if next(gen, "done") == "done":
                break
            step += 1
        for mm in pv_prev[16 * 6 :]:
            mm()
        if prev:
            epilogue(prev[0], prev[1], prev_y)
        prev = (pair, qc, ax_tile)
    # drain the last block
    last_y = alloc_y()
    for mm in pv_mms(prev[0], prev[1], prev[2], last_y):
        mm()
    epilogue(prev[0], prev[1], last_y)


def _build():
    if "nc" in _compiled:
        return _compiled["nc"]
    nc = bacc.Bacc("TRN2", target_bir_lowering=False, debug=False,
                   num_devices=N_CORES)
    from contextlib import ExitStack
    with tile.TileContext(nc) as tc, ExitStack() as ctx:
        _emit(tc, ctx)
    nc.compile()
    _compiled["nc"] = nc
    return nc


def kernel(x, Wq, bq, Wk, bk, Wv, bv, _profile=False):
    x = np.asarray(x, dtype=np.float32)
    Wq = np.asarray(Wq, dtype=np.float32)
    Wk = np.asarray(Wk, dtype=np.float32)
    Wv = np.asarray(Wv, dtype=np.float32)
    bq = np.asarray(bq, dtype=np.float32)
    bk = np.asarray(bk, dtype=np.float32)
    bv = np.asarray(bv, dtype=np.float32)

    nc = _build()

    bf = ml_dtypes.bfloat16
    xT = [np.ascontiguousarray(x[b].T).astype(bf) for b in range(B)]
    in_maps = []
    for c in range(N_CORES):
        b, g = divmod(c, HPC)
        sl = slice(g * C, (g + 1) * C)
        in_maps.append({
            "xT": xT[b],
            "wq": np.ascontiguousarray(Wq[:, sl]).astype(bf),
            "wk": np.ascontiguousarray(Wk[:, sl]).astype(bf),
            "wv": np.ascontiguousarray(Wv[:, sl]).astype(bf),
            "bq": np.ascontiguousarray(bq[sl]),
            "bk": np.ascontiguousarray(bk[sl]),
            "bv": np.ascontiguousarray(bv[sl]),
        })

    from concourse.bass_utils import run_bass_kernel_spmd

    if _profile:
        _install_ntff_hook()
    res = run_bass_kernel_spmd(nc, in_maps, list(range(N_CORES)),
                               trace=_profile)
    out = np.empty((B, S, D), dtype=np.float32)
    for c in range(N_CORES):
        b, g = divmod(c, HPC)
        out[b, :, g * C : (g + 1) * C] = res.results[c]["y"]
    if _profile:
        kernel.last_exec_time_ns = res.exec_time_ns
    return out


# revision 17
# speedup vs baseline: 1.0117x; 1.0117x over previous
"""Trainium2 Bass kernel for nn_CausalSelfAttention_22127671509246.

Full (unsharded) inputs in, full output out. Internally shards across 8
NeuronCores: core c handles batch b = c // 4 and head group g = c % 4
(heads 4g..4g+3, i.e. a 256-wide slice of the QKV output channels).

Per-core compute (all matmuls bf16, f32 PSUM accumulation):
  - Q^T, K^T projections in channel-major layout [256, 2048]
  - V projection in row-major layout with a ones column appended per head
    (so the PV matmul also produces the softmax denominator)
  - attention processed in head PAIRS (partition bases 0 and 64) so the
    K=64 QK matmuls overlap in distinct PE row groups
  - attT[k, q] = K^T_h.T @ Q^T_h -> exp(attT / 8) -> SBUF-resident bf16 ax
    buffer -> PV accumulated per 128-row q tile over all k tiles ->
    normalize by the ones-column denominator.

Schedule (v2): the kernel is paced by the PE; ScalarE alone cannot keep up
with the 16.8M-element exp stream (would be ~130us), so exp tiles are split
between ScalarE (activation Exp) and the DVE (Schraudolph bit-trick:
int16 = logit * 16/ln2 + (127*128 - C), whose bit pattern IS bf16 exp;
~1.8% rms on a ~37% slice of tiles).  Projections are not a separate
phase: K/Q ct0 are pipelined against the DMA at the start so attention
begins at ~5us, and all remaining projection matmuls are stuffed into the
attention blocks' PE stream.  This keeps the PE dense end-to-end so the
HAM clock gate stays at 8/8 (2.4 GHz) instead of oscillating.
Softmax max-subtraction is skipped: logits are ~N(0,1) (max |logit| ~ 7),
so exp never overflows in f32 and softmax is shift-invariant.
"""

import os
import sys
import types

sys.path.insert(0, "/opt/trn_rl_repo")

import numpy as np
import ml_dtypes

import concourse.bass as bass
import concourse.bacc as bacc
import concourse.mybir as mybir
import concourse.tile as tile
from concourse.bass import ts

B, S, D = 2, 2048, 1024
H, HD = 16, 64
N_CORES = 8
C = 256           # output channels per core (4 heads)
CT = C // 128     # channel tiles per core
KD = D // 128     # contraction chunks for the projections
SC = S // 512     # 512-wide column chunks of S
STL = S // 128    # 128-row tiles of S
HPC = 4           # heads per core
SCALE = 1.0 / np.sqrt(HD)

# Schraudolph exp on DVE: i16 = raw_logit * (SCALE*128/ln2) + (127*128 - C0)
SCH_C0 = 7.5
SCH_A = float(SCALE * 128.0 / np.log(2.0))
SCH_B = float(127.0 * 128.0 - SCH_C0)
# kt tiles handled by the DVE (rest on ScalarE); spread so each kp step
# keeps both engines fed.
DVE_KT = frozenset((1, 4, 6, 9, 11, 14))
DVE_KT_LATE = frozenset((1, 3, 5, 7, 9, 11, 13, 15))

F32 = mybir.dt.float32
BF16 = mybir.dt.bfloat16
I16 = mybir.dt.int16

_compiled = {}


def _install_ntff_hook():
    """Optional: register the axon NTFF profiling hook if the image lacks it."""
    if "antenv.axon_hooks" in sys.modules:
        return
    try:
        import trn_agent_boot.trn_boot as tb

        mod = types.ModuleType("antenv.axon_hooks")
        hook = tb._ntff_profile_via_ctypes("/opt/axon/libaxon_pjrt.so")
        mod.get_axon_ntff_profile_hook = lambda: hook
        mod.set_axon_ntff_profile_hook = lambda h: None
        sys.modules["antenv.axon_hooks"] = mod
    except Exception:
        pass


def _emit(tc, ctx):
    nc = tc.nc
    xT = nc.dram_tensor("xT", [D, S], BF16, kind="ExternalInput").ap()
    wq = nc.dram_tensor("wq", [D, C], BF16, kind="ExternalInput").ap()
    wk = nc.dram_tensor("wk", [D, C], BF16, kind="ExternalInput").ap()
    wv = nc.dram_tensor("wv", [D, C], BF16, kind="ExternalInput").ap()
    bq = nc.dram_tensor("bq", [C], F32, kind="ExternalInput").ap()
    bk = nc.dram_tensor("bk", [C], F32, kind="ExternalInput").ap()
    bv = nc.dram_tensor("bv", [C], F32, kind="ExternalInput").ap()
    y = nc.dram_tensor("y", [S, C], F32, kind="ExternalOutput").ap()

    singles = ctx.enter_context(tc.tile_pool(name="singles", bufs=1))
    ax_pool = ctx.enter_context(tc.tile_pool(name="ax", bufs=3))
    yout_pool = ctx.enter_context(tc.tile_pool(name="yout", bufs=3))
    recip_pool = ctx.enter_context(tc.tile_pool(name="recip", bufs=4))
    ps_pool = ctx.enter_context(tc.tile_pool(name="ps", bufs=3, space="PSUM"))
    psy_pool = ctx.enter_context(tc.tile_pool(name="psy", bufs=1, space="PSUM"))

    # ---- SBUF tiles ----
    xT_r = xT.rearrange("(o p) s -> p o s", p=128)
    xT_sb = singles.tile([128, KD, S], BF16)
    w_sbs = {}
    w_sbs["k"] = singles.tile([128, KD, C], BF16, tag="wk", name="wk_sb")
    w_sbs["q"] = singles.tile([128, KD, C], BF16, tag="wq", name="wq_sb")
    w_sbs["v"] = singles.tile([128, KD, C], BF16, tag="wv", name="wv_sb")
    bq_sb = singles.tile([128, CT], F32, tag="bq")
    bk_sb = singles.tile([128, CT], F32, tag="bk")
    wq_r = wq.rearrange("(o p) c -> p o c", p=128)
    wk_r = wk.rearrange("(o p) c -> p o c", p=128)
    wv_r = wv.rearrange("(o p) c -> p o c", p=128)

    # DMA order = arrival order on the sync queue.  Weights go as single
    # full-tensor transfers (4KB per partition line = full DMA bandwidth);
    # xT sc0 as small per-kd chunks (latency: the prologue projections
    # consume them one by one), the rest of xT as per-kd 1536-wide chunks
    # (3KB lines).
    nc.sync.dma_start(w_sbs["k"][:], wk_r)
    nc.sync.dma_start(bk_sb[:], bk.rearrange("(o p) -> p o", p=128))
    for kd in range(KD):
        nc.sync.dma_start(xT_sb[:, kd, 0:512], xT_r[:, kd, 0:512])
        if kd == 1:
            nc.sync.dma_start(w_sbs["q"][:], wq_r)
            nc.sync.dma_start(bq_sb[:], bq.rearrange("(o p) -> p o", p=128))
    nc.sync.dma_start(w_sbs["v"][:], wv_r)
    for kd in range(KD):
        nc.sync.dma_start(xT_sb[:, kd, 512:2048], xT_r[:, kd, 512:2048])
    # bv broadcast across partitions (DMA with partition step 0)
    bv_bc = singles.tile([128, C], F32, tag="bvbc")
    bv_bcast_ap = bass.AP(tensor=bv.tensor, offset=bv.offset,
                          ap=[[0, 128]] + list(bv.ap))
    nc.gpsimd.dma_start(out=bv_bc[:], in_=bv_bcast_ap)

    # V with a ones column appended per head: [128, s_tile, head, 65]
    v_sb = singles.tile([128, STL, HPC, HD + 1], BF16, tag="vones")
    nc.vector.memset(v_sb[:, :, :, HD], 1.0)

    qt_sb = singles.tile([128, CT, S], BF16, tag="qt")
    kt_sb = singles.tile([128, CT, S], BF16, tag="kt")

    # HAM warmup: junk matmuls on a zeroed scratch tile into the (not yet
    # used) y0 PSUM bank.  They have no DMA deps, so they keep the PE busy
    # while the prologue projections wait on input DMA — otherwise the PE
    # idles in ~3us chunks and the clock gate holds it at 1.2 GHz for the
    # first ~50us.
    warm_sb = singles.tile([128, 512], BF16, tag="warm")
    nc.vector.memset(warm_sb[:], 0.0)
    warm_ps = psy_pool.tile([128, 4, HD + 1], F32, tag="y0", name="warm_ps")

    def dummies(n):
        for _ in range(n):
            nc.tensor.matmul(
                warm_ps[:], lhsT=warm_sb[:, 0:128], rhs=warm_sb[:, 0:260],
                start=True, stop=True,
            )

    # ---- projection groups (8 matmuls + 1 bias op each) ----
    def proj_qk(which, ct, sc):
        w_sb = w_sbs[which]
        dst = qt_sb if which == "q" else kt_sb
        bias = bq_sb if which == "q" else bk_sb
        ps = ps_pool.tile([128, 1024], F32, tag="qk", name="ps_proj")
        for kd in range(KD):
            nc.tensor.matmul(
                ps[:, 0:512],
                lhsT=w_sb[:, kd, ts(ct, 128)],
                rhs=xT_sb[:, kd, ts(sc, 512)],
                start=(kd == 0),
                stop=(kd == KD - 1),
            )
        nc.vector.tensor_scalar_add(
            dst[:, ct, ts(sc, 512)], ps[:, 0:512], bias[:, ct : ct + 1]
        )

    def proj_v(st):
        # V projections run only in block 0, when the y PSUM banks are
        # still idle — use them instead of stealing a QK-ring slot (the
        # ring slot would be held until the bias-add clears the DVE FIFO,
        # stalling QK allocation and starving the exp engines).
        ps = psy_pool.tile([128, 4, HD + 1], F32, tag=f"y{st % 2}",
                           name="vps")
        flat = ps[:].rearrange("p a b -> p (a b)")
        for kd in range(KD):
            nc.tensor.matmul(
                flat[:, 0:C],
                lhsT=xT_sb[:, kd, ts(st, 128)],
                rhs=w_sbs["v"][:, kd, :],
                start=(kd == 0),
                stop=(kd == KD - 1),
            )
        nc.vector.tensor_tensor(
            v_sb[:, st, :, 0:HD],
            flat[:, 0:C].rearrange("p (h d) -> p h d", h=HPC),
            bv_bc.rearrange("p (h d) -> p h d", h=HPC),
            mybir.AluOpType.add,
        )

    # ---- attention ----
    blocks = [(pair, qc) for pair in range(HPC // 2) for qc in range(SC)]

    def qk_exp_block(pair, qc, ax_tile, pre_steps=(), late=False):
        """Per kp step: 4 QK matmuls (head pair in distinct PE row groups),
        then 2 exps routed to ScalarE or DVE.  pre_steps[s] = closures to
        emit before step s (late projection groups for block 0)."""
        ct = pair
        for kp in range(STL // 2):
            if kp < len(pre_steps):
                for fn in pre_steps[kp]:
                    fn()
            tiles = []
            for u in range(2):
                ps = ps_pool.tile([128, 1024], F32, tag="qk", name="ps_att")
                tiles.append(ps)
            for u in range(2):
                kt = 2 * kp + u
                for hh in range(2):
                    p0 = hh * 64
                    nc.tensor.matmul(
                        tiles[u][:, ts(hh, 512)],
                        lhsT=kt_sb[p0 : p0 + 64, ct, ts(kt, 128)],
                        rhs=qt_sb[p0 : p0 + 64, ct, ts(qc, 512)],
                        start=True,
                        stop=True,
                    )
            for u in range(2):
                kt = 2 * kp + u
                if kt in (DVE_KT_LATE if late else DVE_KT):
                    nc.vector.tensor_scalar(
                        ax_tile[:, kt, :].bitcast(I16), tiles[u][:],
                        SCH_A, SCH_B,
                        mybir.AluOpType.mult, mybir.AluOpType.add,
                    )
                else:
                    nc.scalar.activation(
                        ax_tile[:, kt, :], tiles[u][:],
                        mybir.ActivationFunctionType.Exp, scale=SCALE,
                    )
            yield

    def pv_mms(pair, qc, ax_tile, y_ps):
        """PV matmul closures (hh/j-major, kt innermost: PSUM accumulation
        groups must stay contiguous — interleaving them corrupts)."""
        mms = []
        for hh in range(2):
            h = 2 * pair + hh
            for j in range(4):
                for kt in range(STL):
                    def mm(hh=hh, h=h, j=j, kt=kt):
                        nc.tensor.matmul(
                            y_ps[hh][:, j, :],
                            lhsT=ax_tile[:, kt,
                                         hh * 512 + j * 128
                                         : hh * 512 + (j + 1) * 128],
                            rhs=v_sb[:, kt, h, :],
                            start=(kt == 0),
                            stop=(kt == STL - 1),
                        )
                    mms.append(mm)
        return mms

    def epilogue(pair, qc, y_ps):
        yo = yout_pool.tile([128, 4, 2, HD], F32, tag="yo", name="yo")
        for hh in range(2):
            rc = recip_pool.tile([128, 4], F32, tag="rc", name="rc")
            nc.vector.reciprocal(rc[:], y_ps[hh][:, :, HD])
            nc.vector.tensor_tensor(
                yo[:, :, hh, :],
                y_ps[hh][:, :, 0:HD],
                rc[:, :, None].to_broadcast((128, 4, HD)),
                mybir.AluOpType.mult,
            )
        nc.sync.dma_start(
            y[ts(qc, 512), ts(pair, 2 * HD)].rearrange(
                "(j p) c -> p j c", p=128),
            yo[:].rearrange("p j h d -> p j (h d)"),
        )

    # ---- schedule ----
    # Prologue: K/Q ct0 sc0 so block (0,0) can start immediately; dummy
    # matmuls ahead of each DMA-gated projection matmul keep the PE warm.
    dummies(12)

    def proj_qk_warm(which, ct, sc, per_kd):
        w_sb = w_sbs[which]
        dst = qt_sb if which == "q" else kt_sb
        bias = bq_sb if which == "q" else bk_sb
        ps = ps_pool.tile([128, 1024], F32, tag="qk", name="ps_proj")
        for kd in range(KD):
            dummies(per_kd)
            nc.tensor.matmul(
                ps[:, 0:512],
                lhsT=w_sb[:, kd, ts(ct, 128)],
                rhs=xT_sb[:, kd, ts(sc, 512)],
                start=(kd == 0),
                stop=(kd == KD - 1),
            )
        nc.vector.tensor_scalar_add(
            dst[:, ct, ts(sc, 512)], ps[:, 0:512], bias[:, ct : ct + 1]
        )

    proj_qk_warm("k", 0, 0, 3)
    proj_qk_warm("q", 0, 0, 1)

    # Late projection groups stuffed into the attention blocks' PE stream.
    # Need-by: qt[ct0,sc_j] before block j; V s-tiles 0..3 before block 1
    # step 0, 4..7 before step 2, ... (pv is kt-major, clumps at steps
    # 0/2/4/6); kt[ct1]+qt[ct1,sc0] before block 4; qt[ct1,sc_j] before
    # block 4+j.
    stuff = {}
    # block 0: remaining K ct0 groups ride ahead of their QK consumers
    # (kp step s consumes kt tiles 2s,2s+1 -> sc group (2s+1)//4).
    # V groups use the psy banks, so only proj_qk calls steal a QK-ring
    # slot; keep those <=2 per block and >=4 steps apart.
    stuff[(0, 0)] = [lambda: proj_qk("k", 0, 1)]
    stuff[(0, 2)] = [lambda: proj_qk("k", 0, 2)]
    stuff[(0, 3)] = [lambda: proj_v(0), lambda: proj_v(1),
                     lambda: proj_v(2)]
    stuff[(0, 4)] = [lambda: proj_qk("k", 0, 3),
                     lambda: proj_v(3), lambda: proj_v(4),
                     lambda: proj_v(5)]
    stuff[(0, 5)] = [lambda: proj_v(6), lambda: proj_v(7),
                     lambda: proj_v(8)]
    stuff[(0, 6)] = [lambda: proj_v(9), lambda: proj_v(10),
                     lambda: proj_v(11), lambda: proj_v(12)]
    stuff[(0, 7)] = [lambda: proj_qk("q", 0, 1),
                     lambda: proj_v(13), lambda: proj_v(14),
                     lambda: proj_v(15)]
    stuff[(1, 2)] = [lambda: proj_qk("q", 0, 2)]
    stuff[(1, 6)] = [lambda: proj_qk("k", 1, 0)]
    stuff[(2, 1)] = [lambda: proj_qk("q", 0, 3)]
    stuff[(2, 5)] = [lambda: proj_qk("k", 1, 1)]
    stuff[(3, 1)] = [lambda: proj_qk("k", 1, 2)]
    stuff[(3, 5)] = [lambda: proj_qk("q", 1, 0)]
    stuff[(4, 1)] = [lambda: proj_qk("k", 1, 3)]
    stuff[(4, 5)] = [lambda: proj_qk("q", 1, 1)]
    stuff[(5, 1)] = [lambda: proj_qk("q", 1, 2)]
    stuff[(6, 1)] = [lambda: proj_qk("q", 1, 3)]

    def alloc_y():
        return [psy_pool.tile([128, 4, HD + 1], F32, tag=f"y{hh}",
                              name=f"y{hh}") for hh in range(2)]

    prev = None  # (pair, qc, ax_tile)
    prev_y = None
    for i, (pair, qc) in enumerate(blocks):
        ax_tile = ax_pool.tile([128, STL, 1024], BF16, tag="ax", name="ax")
        if prev:
            prev_y = alloc_y()
            pv_prev = pv_mms(prev[0], prev[1], prev[2], prev_y)
        else:
            pv_prev = []
        assert len(pv_prev) in (0, 128)
        step = 0
        gen = qk_exp_block(pair, qc, ax_tile, late=(i == len(blocks) - 1))
        while True:
            for fn in stuff.get((i, step), ()):
                fn()
            if i == 0 and step < 8:
                dummies(2)  # block 0 is DMA-paced; keep the PE warm
            # PV matmuls of the previous block, 32 per two kp steps,
            # starting at step 2 so the previous block's trailing exps
            # have slack (the first PV group reads every kt tile).
            if step in (2, 4, 6):
                for mm in pv_prev[16 * (step - 2) : 16 * step]:
                    mm()
            if next(gen, "done") == "done":
                break
            step += 1
        for mm in pv_prev[16 * 6 :]:
            mm()
        if prev:
            epilogue(prev[0], prev[1], prev_y)
        prev = (pair, qc, ax_tile)
    # drain the last block
    last_y = alloc_y()
    for mm in pv_mms(prev[0], prev[1], prev[2], last_y):
        mm()
    epilogue(prev[0], prev[1], last_y)


def _build():
    if "nc" in _compiled:
        return _compiled["nc"]
    nc = bacc.Bacc("TRN2", target_bir_lowering=False, debug=False,
                   num_devices=N_CORES)
    from contextlib import ExitStack
    with tile.TileContext(nc) as tc, ExitStack() as ctx:
        _emit(tc, ctx)
    nc.compile()
    _compiled["nc"] = nc
    return nc


def kernel(x, Wq, bq, Wk, bk, Wv, bv, _profile=False):
    x = np.asarray(x, dtype=np.float32)
    Wq = np.asarray(Wq, dtype=np.float32)
    Wk = np.asarray(Wk, dtype=np.float32)
    Wv = np.asarray(Wv, dtype=np.float32)
    bq = np.asarray(bq, dtype=np.float32)
    bk = np.asarray(bk, dtype=np.float32)
    bv = np.asarray(bv, dtype=np.float32)

    nc = _build()

    bf = ml_dtypes.bfloat16
    xT = [np.ascontiguousarray(x[b].T).astype(bf) for b in range(B)]
    in_maps = []
    for c in range(N_CORES):
        b, g = divmod(c, HPC)
        sl = slice(g * C, (g + 1) * C)
        in_maps.append({
            "xT": xT[b],
            "wq": np.ascontiguousarray(Wq[:, sl]).astype(bf),
            "wk": np.ascontiguousarray(Wk[:, sl]).astype(bf),
            "wv": np.ascontiguousarray(Wv[:, sl]).astype(bf),
            "bq": np.ascontiguousarray(bq[sl]),
            "bk": np.ascontiguousarray(bk[sl]),
            "bv": np.ascontiguousarray(bv[sl]),
        })

    from concourse.bass_utils import run_bass_kernel_spmd

    if _profile:
        _install_ntff_hook()
    res = run_bass_kernel_spmd(nc, in_maps, list(range(N_CORES)),
                               trace=_profile)
    out = np.empty((B, S, D), dtype=np.float32)
    for c in range(N_CORES):
        b, g = divmod(c, HPC)
        out[b, :, g * C : (g + 1) * C] = res.results[c]["y"]
    if _profile:
        kernel.last_exec_time_ns = res.exec_time_ns
    return out


# revision 18
# speedup vs baseline: 1.0206x; 1.0088x over previous
"""Trainium2 Bass kernel for nn_CausalSelfAttention_22127671509246.

Full (unsharded) inputs in, full output out. Internally shards across 8
NeuronCores: core c handles batch b = c // 4 and head group g = c % 4
(heads 4g..4g+3, i.e. a 256-wide slice of the QKV output channels).

Per-core compute (all matmuls bf16, f32 PSUM accumulation):
  - Q^T, K^T projections in channel-major layout [256, 2048]
  - V projection in row-major layout with a ones column appended per head
    (so the PV matmul also produces the softmax denominator)
  - attention processed in head PAIRS (partition bases 0 and 64) so the
    K=64 QK matmuls overlap in distinct PE row groups
  - attT[k, q] = K^T_h.T @ Q^T_h -> exp(attT / 8) -> SBUF-resident bf16 ax
    buffer -> PV accumulated per 128-row q tile over all k tiles ->
    normalize by the ones-column denominator.

Schedule (v2): the kernel is paced by the PE; ScalarE alone cannot keep up
with the 16.8M-element exp stream (would be ~130us), so exp tiles are split
between ScalarE (activation Exp) and the DVE (Schraudolph bit-trick:
int16 = logit * 16/ln2 + (127*128 - C), whose bit pattern IS bf16 exp;
~1.8% rms on a ~37% slice of tiles).  Projections are not a separate
phase: K/Q ct0 are pipelined against the DMA at the start so attention
begins at ~5us, and all remaining projection matmuls are stuffed into the
attention blocks' PE stream.  This keeps the PE dense end-to-end so the
HAM clock gate stays at 8/8 (2.4 GHz) instead of oscillating.
Softmax max-subtraction is skipped: logits are ~N(0,1) (max |logit| ~ 7),
so exp never overflows in f32 and softmax is shift-invariant.
"""

import os
import sys
import types

sys.path.insert(0, "/opt/trn_rl_repo")

import numpy as np
import ml_dtypes

import concourse.bass as bass
import concourse.bacc as bacc
import concourse.mybir as mybir
import concourse.tile as tile
from concourse.bass import ts

B, S, D = 2, 2048, 1024
H, HD = 16, 64
N_CORES = 8
C = 256           # output channels per core (4 heads)
CT = C // 128     # channel tiles per core
KD = D // 128     # contraction chunks for the projections
SC = S // 512     # 512-wide column chunks of S
STL = S // 128    # 128-row tiles of S
HPC = 4           # heads per core
SCALE = 1.0 / np.sqrt(HD)

# Schraudolph exp on DVE: i16 = raw_logit * (SCALE*128/ln2) + (127*128 - C0)
SCH_C0 = 7.5
SCH_A = float(SCALE * 128.0 / np.log(2.0))
SCH_B = float(127.0 * 128.0 - SCH_C0)
# kt tiles handled by the DVE (rest on ScalarE); spread so each kp step
# keeps both engines fed.
DVE_KT = frozenset((1, 3, 5, 8, 10, 12, 14))
DVE_KT_LATE = frozenset((1, 3, 5, 7, 9, 11, 13))

F32 = mybir.dt.float32
BF16 = mybir.dt.bfloat16
I16 = mybir.dt.int16

_compiled = {}


def _install_ntff_hook():
    """Optional: register the axon NTFF profiling hook if the image lacks it."""
    if "antenv.axon_hooks" in sys.modules:
        return
    try:
        import trn_agent_boot.trn_boot as tb

        mod = types.ModuleType("antenv.axon_hooks")
        hook = tb._ntff_profile_via_ctypes("/opt/axon/libaxon_pjrt.so")
        mod.get_axon_ntff_profile_hook = lambda: hook
        mod.set_axon_ntff_profile_hook = lambda h: None
        sys.modules["antenv.axon_hooks"] = mod
    except Exception:
        pass


def _emit(tc, ctx):
    nc = tc.nc
    xT = nc.dram_tensor("xT", [D, S], BF16, kind="ExternalInput").ap()
    wq = nc.dram_tensor("wq", [D, C], BF16, kind="ExternalInput").ap()
    wk = nc.dram_tensor("wk", [D, C], BF16, kind="ExternalInput").ap()
    wv = nc.dram_tensor("wv", [D, C], BF16, kind="ExternalInput").ap()
    bq = nc.dram_tensor("bq", [C], F32, kind="ExternalInput").ap()
    bk = nc.dram_tensor("bk", [C], F32, kind="ExternalInput").ap()
    bv = nc.dram_tensor("bv", [C], F32, kind="ExternalInput").ap()
    y = nc.dram_tensor("y", [S, C], F32, kind="ExternalOutput").ap()

    singles = ctx.enter_context(tc.tile_pool(name="singles", bufs=1))
    ax_pool = ctx.enter_context(tc.tile_pool(name="ax", bufs=3))
    yout_pool = ctx.enter_context(tc.tile_pool(name="yout", bufs=3))
    recip_pool = ctx.enter_context(tc.tile_pool(name="recip", bufs=4))
    ps_pool = ctx.enter_context(tc.tile_pool(name="ps", bufs=3, space="PSUM"))
    psy_pool = ctx.enter_context(tc.tile_pool(name="psy", bufs=1, space="PSUM"))

    # ---- SBUF tiles ----
    xT_r = xT.rearrange("(o p) s -> p o s", p=128)
    xT_sb = singles.tile([128, KD, S], BF16)
    w_sbs = {}
    w_sbs["k"] = singles.tile([128, KD, C], BF16, tag="wk", name="wk_sb")
    w_sbs["q"] = singles.tile([128, KD, C], BF16, tag="wq", name="wq_sb")
    w_sbs["v"] = singles.tile([128, KD, C], BF16, tag="wv", name="wv_sb")
    bq_sb = singles.tile([128, CT], F32, tag="bq")
    bk_sb = singles.tile([128, CT], F32, tag="bk")
    wq_r = wq.rearrange("(o p) c -> p o c", p=128)
    wk_r = wk.rearrange("(o p) c -> p o c", p=128)
    wv_r = wv.rearrange("(o p) c -> p o c", p=128)

    # DMA order = arrival order on the sync queue.  Weights go as single
    # full-tensor transfers (4KB per partition line = full DMA bandwidth);
    # xT sc0 as small per-kd chunks (latency: the prologue projections
    # consume them one by one), the rest of xT as per-kd 1536-wide chunks
    # (3KB lines).
    nc.sync.dma_start(w_sbs["k"][:], wk_r)
    nc.sync.dma_start(bk_sb[:], bk.rearrange("(o p) -> p o", p=128))
    for kd in range(KD):
        nc.sync.dma_start(xT_sb[:, kd, 0:512], xT_r[:, kd, 0:512])
        if kd == 1:
            nc.sync.dma_start(w_sbs["q"][:], wq_r)
            nc.sync.dma_start(bq_sb[:], bq.rearrange("(o p) -> p o", p=128))
    nc.sync.dma_start(w_sbs["v"][:], wv_r)
    for kd in range(KD):
        nc.sync.dma_start(xT_sb[:, kd, 512:2048], xT_r[:, kd, 512:2048])
    # bv broadcast across partitions (DMA with partition step 0)
    bv_bc = singles.tile([128, C], F32, tag="bvbc")
    bv_bcast_ap = bass.AP(tensor=bv.tensor, offset=bv.offset,
                          ap=[[0, 128]] + list(bv.ap))
    nc.gpsimd.dma_start(out=bv_bc[:], in_=bv_bcast_ap)

    # V with a ones column appended per head: [128, s_tile, head, 65]
    v_sb = singles.tile([128, STL, HPC, HD + 1], BF16, tag="vones")
    nc.vector.memset(v_sb[:, :, :, HD], 1.0)

    qt_sb = singles.tile([128, CT, S], BF16, tag="qt")
    kt_sb = singles.tile([128, CT, S], BF16, tag="kt")

    # HAM warmup: junk matmuls on a zeroed scratch tile into the (not yet
    # used) y0 PSUM bank.  They have no DMA deps, so they keep the PE busy
    # while the prologue projections wait on input DMA — otherwise the PE
    # idles in ~3us chunks and the clock gate holds it at 1.2 GHz for the
    # first ~50us.
    warm_sb = singles.tile([128, 512], BF16, tag="warm")
    nc.vector.memset(warm_sb[:], 0.0)
    warm_ps = psy_pool.tile([128, 4, HD + 1], F32, tag="y0", name="warm_ps")

    def dummies(n):
        for _ in range(n):
            nc.tensor.matmul(
                warm_ps[:], lhsT=warm_sb[:, 0:128], rhs=warm_sb[:, 0:260],
                start=True, stop=True,
            )

    # ---- projection groups (8 matmuls + 1 bias op each) ----
    def proj_qk(which, ct, sc):
        w_sb = w_sbs[which]
        dst = qt_sb if which == "q" else kt_sb
        bias = bq_sb if which == "q" else bk_sb
        ps = ps_pool.tile([128, 1024], F32, tag="qk", name="ps_proj")
        for kd in range(KD):
            nc.tensor.matmul(
                ps[:, 0:512],
                lhsT=w_sb[:, kd, ts(ct, 128)],
                rhs=xT_sb[:, kd, ts(sc, 512)],
                start=(kd == 0),
                stop=(kd == KD - 1),
            )
        nc.vector.tensor_scalar_add(
            dst[:, ct, ts(sc, 512)], ps[:, 0:512], bias[:, ct : ct + 1]
        )

    def proj_v(st):
        # V projections run only in block 0, when the y PSUM banks are
        # still idle — use them instead of stealing a QK-ring slot (the
        # ring slot would be held until the bias-add clears the DVE FIFO,
        # stalling QK allocation and starving the exp engines).
        ps = psy_pool.tile([128, 4, HD + 1], F32, tag=f"y{st % 2}",
                           name="vps")
        flat = ps[:].rearrange("p a b -> p (a b)")
        for kd in range(KD):
            nc.tensor.matmul(
                flat[:, 0:C],
                lhsT=xT_sb[:, kd, ts(st, 128)],
                rhs=w_sbs["v"][:, kd, :],
                start=(kd == 0),
                stop=(kd == KD - 1),
            )
        nc.vector.tensor_tensor(
            v_sb[:, st, :, 0:HD],
            flat[:, 0:C].rearrange("p (h d) -> p h d", h=HPC),
            bv_bc.rearrange("p (h d) -> p h d", h=HPC),
            mybir.AluOpType.add,
        )

    # ---- attention ----
    blocks = [(pair, qc) for pair in range(HPC // 2) for qc in range(SC)]

    def qk_exp_block(pair, qc, ax_tile, pre_steps=(), late=False):
        """Per kp step: 4 QK matmuls (head pair in distinct PE row groups),
        then 2 exps routed to ScalarE or DVE.  pre_steps[s] = closures to
        emit before step s (late projection groups for block 0)."""
        ct = pair
        for kp in range(STL // 2):
            if kp < len(pre_steps):
                for fn in pre_steps[kp]:
                    fn()
            tiles = []
            for u in range(2):
                ps = ps_pool.tile([128, 1024], F32, tag="qk", name="ps_att")
                tiles.append(ps)
            for u in range(2):
                kt = 2 * kp + u
                for hh in range(2):
                    p0 = hh * 64
                    nc.tensor.matmul(
                        tiles[u][:, ts(hh, 512)],
                        lhsT=kt_sb[p0 : p0 + 64, ct, ts(kt, 128)],
                        rhs=qt_sb[p0 : p0 + 64, ct, ts(qc, 512)],
                        start=True,
                        stop=True,
                    )
            for u in range(2):
                kt = 2 * kp + u
                if kt in (DVE_KT_LATE if late else DVE_KT):
                    nc.vector.tensor_scalar(
                        ax_tile[:, kt, :].bitcast(I16), tiles[u][:],
                        SCH_A, SCH_B,
                        mybir.AluOpType.mult, mybir.AluOpType.add,
                    )
                else:
                    nc.scalar.activation(
                        ax_tile[:, kt, :], tiles[u][:],
                        mybir.ActivationFunctionType.Exp, scale=SCALE,
                    )
            yield

    def pv_mms(pair, qc, ax_tile, y_ps):
        """PV matmul closures (hh/j-major, kt innermost: PSUM accumulation
        groups must stay contiguous — interleaving them corrupts)."""
        mms = []
        for hh in range(2):
            h = 2 * pair + hh
            for j in range(4):
                for kt in range(STL):
                    def mm(hh=hh, h=h, j=j, kt=kt):
                        nc.tensor.matmul(
                            y_ps[hh][:, j, :],
                            lhsT=ax_tile[:, kt,
                                         hh * 512 + j * 128
                                         : hh * 512 + (j + 1) * 128],
                            rhs=v_sb[:, kt, h, :],
                            start=(kt == 0),
                            stop=(kt == STL - 1),
                        )
                    mms.append(mm)
        return mms

    def epilogue(pair, qc, y_ps):
        yo = yout_pool.tile([128, 4, 2, HD], F32, tag="yo", name="yo")
        for hh in range(2):
            rc = recip_pool.tile([128, 4], F32, tag="rc", name="rc")
            nc.vector.reciprocal(rc[:], y_ps[hh][:, :, HD])
            nc.vector.tensor_tensor(
                yo[:, :, hh, :],
                y_ps[hh][:, :, 0:HD],
                rc[:, :, None].to_broadcast((128, 4, HD)),
                mybir.AluOpType.mult,
            )
        nc.sync.dma_start(
            y[ts(qc, 512), ts(pair, 2 * HD)].rearrange(
                "(j p) c -> p j c", p=128),
            yo[:].rearrange("p j h d -> p j (h d)"),
        )

    # ---- schedule ----
    # Prologue: K/Q ct0 sc0 so block (0,0) can start immediately; dummy
    # matmuls ahead of each DMA-gated projection matmul keep the PE warm.
    dummies(12)

    def proj_qk_warm(which, ct, sc, per_kd):
        w_sb = w_sbs[which]
        dst = qt_sb if which == "q" else kt_sb
        bias = bq_sb if which == "q" else bk_sb
        ps = ps_pool.tile([128, 1024], F32, tag="qk", name="ps_proj")
        for kd in range(KD):
            dummies(per_kd)
            nc.tensor.matmul(
                ps[:, 0:512],
                lhsT=w_sb[:, kd, ts(ct, 128)],
                rhs=xT_sb[:, kd, ts(sc, 512)],
                start=(kd == 0),
                stop=(kd == KD - 1),
            )
        nc.vector.tensor_scalar_add(
            dst[:, ct, ts(sc, 512)], ps[:, 0:512], bias[:, ct : ct + 1]
        )

    proj_qk_warm("k", 0, 0, 3)
    proj_qk_warm("q", 0, 0, 1)

    # Late projection groups stuffed into the attention blocks' PE stream.
    # Need-by: qt[ct0,sc_j] before block j; V s-tiles 0..3 before block 1
    # step 0, 4..7 before step 2, ... (pv is kt-major, clumps at steps
    # 0/2/4/6); kt[ct1]+qt[ct1,sc0] before block 4; qt[ct1,sc_j] before
    # block 4+j.
    stuff = {}
    # block 0: remaining K ct0 groups ride ahead of their QK consumers
    # (kp step s consumes kt tiles 2s,2s+1 -> sc group (2s+1)//4).
    # V groups use the psy banks, so only proj_qk calls steal a QK-ring
    # slot; keep those <=2 per block and >=4 steps apart.
    stuff[(0, 0)] = [lambda: proj_qk("k", 0, 1)]
    stuff[(0, 2)] = [lambda: proj_qk("k", 0, 2)]
    stuff[(0, 3)] = [lambda: proj_v(0), lambda: proj_v(1),
                     lambda: proj_v(2)]
    stuff[(0, 4)] = [lambda: proj_qk("k", 0, 3),
                     lambda: proj_v(3), lambda: proj_v(4),
                     lambda: proj_v(5)]
    stuff[(0, 5)] = [lambda: proj_v(6), lambda: proj_v(7),
                     lambda: proj_v(8)]
    stuff[(0, 6)] = [lambda: proj_v(9), lambda: proj_v(10),
                     lambda: proj_v(11), lambda: proj_v(12)]
    stuff[(0, 7)] = [lambda: proj_qk("q", 0, 1),
                     lambda: proj_v(13), lambda: proj_v(14),
                     lambda: proj_v(15)]
    stuff[(1, 2)] = [lambda: proj_qk("q", 0, 2)]
    stuff[(1, 6)] = [lambda: proj_qk("k", 1, 0)]
    stuff[(2, 1)] = [lambda: proj_qk("q", 0, 3)]
    stuff[(2, 5)] = [lambda: proj_qk("k", 1, 1)]
    stuff[(3, 1)] = [lambda: proj_qk("k", 1, 2)]
    stuff[(3, 5)] = [lambda: proj_qk("q", 1, 0)]
    stuff[(4, 1)] = [lambda: proj_qk("k", 1, 3)]
    stuff[(4, 5)] = [lambda: proj_qk("q", 1, 1)]
    stuff[(5, 1)] = [lambda: proj_qk("q", 1, 2)]
    stuff[(6, 1)] = [lambda: proj_qk("q", 1, 3)]

    def alloc_y():
        return [psy_pool.tile([128, 4, HD + 1], F32, tag=f"y{hh}",
                              name=f"y{hh}") for hh in range(2)]

    prev = None  # (pair, qc, ax_tile)
    prev_y = None
    for i, (pair, qc) in enumerate(blocks):
        ax_tile = ax_pool.tile([128, STL, 1024], BF16, tag="ax", name="ax")
        if prev:
            prev_y = alloc_y()
            pv_prev = pv_mms(prev[0], prev[1], prev[2], prev_y)
        else:
            pv_prev = []
        assert len(pv_prev) in (0, 128)
        step = 0
        gen = qk_exp_block(pair, qc, ax_tile, late=(i == len(blocks) - 1))
        while True:
            for fn in stuff.get((i, step), ()):
                fn()
            if i == 0 and step < 8:
                dummies(2)  # block 0 is DMA-paced; keep the PE warm
            # PV matmuls of the previous block, 32 per two kp steps,
            # starting at step 2 so the previous block's trailing exps
            # have slack (the first PV group reads every kt tile).
            if step in (2, 4, 6):
                for mm in pv_prev[16 * (step - 2) : 16 * step]:
                    mm()
            if next(gen, "done") == "done":
                break
            step += 1
        for mm in pv_prev[16 * 6 :]:
            mm()
        if prev:
            epilogue(prev[0], prev[1], prev_y)
        prev = (pair, qc, ax_tile)
    # drain the last block
    last_y = alloc_y()
    for mm in pv_mms(prev[0], prev[1], prev[2], last_y):
        mm()
    epilogue(prev[0], prev[1], last_y)


def _build():
    if "nc" in _compiled:
        return _compiled["nc"]
    nc = bacc.Bacc("TRN2", target_bir_lowering=False, debug=False,
                   num_devices=N_CORES)
    from contextlib import ExitStack
    with tile.TileContext(nc) as tc, ExitStack() as ctx:
        _emit(tc, ctx)
    nc.compile()
    _compiled["nc"] = nc
    return nc


def kernel(x, Wq, bq, Wk, bk, Wv, bv, _profile=False):
    x = np.asarray(x, dtype=np.float32)
    Wq = np.asarray(Wq, dtype=np.float32)
    Wk = np.asarray(Wk, dtype=np.float32)
    Wv = np.asarray(Wv, dtype=np.float32)
    bq = np.asarray(bq, dtype=np.float32)
    bk = np.asarray(bk, dtype=np.float32)
    bv = np.asarray(bv, dtype=np.float32)

    nc = _build()

    bf = ml_dtypes.bfloat16
    xT = [np.ascontiguousarray(x[b].T).astype(bf) for b in range(B)]
    in_maps = []
    for c in range(N_CORES):
        b, g = divmod(c, HPC)
        sl = slice(g * C, (g + 1) * C)
        in_maps.append({
            "xT": xT[b],
            "wq": np.ascontiguousarray(Wq[:, sl]).astype(bf),
            "wk": np.ascontiguousarray(Wk[:, sl]).astype(bf),
            "wv": np.ascontiguousarray(Wv[:, sl]).astype(bf),
            "bq": np.ascontiguousarray(bq[sl]),
            "bk": np.ascontiguousarray(bk[sl]),
            "bv": np.ascontiguousarray(bv[sl]),
        })

    from concourse.bass_utils import run_bass_kernel_spmd

    if _profile:
        _install_ntff_hook()
    res = run_bass_kernel_spmd(nc, in_maps, list(range(N_CORES)),
                               trace=_profile)
    out = np.empty((B, S, D), dtype=np.float32)
    for c in range(N_CORES):
        b, g = divmod(c, HPC)
        out[b, :, g * C : (g + 1) * C] = res.results[c]["y"]
    if _profile:
        kernel.last_exec_time_ns = res.exec_time_ns
    return out
